# revision 3
# baseline (speedup 1.0000x reference)
"""IoU loss kernel for Trainium2, data-parallel over the batch dim on 8 cores.

Math (per reference):
    probs = softmax(inputs, axis=1)                       # (8, 13, 800, 800)
    intersection = sum_pix probs[b, t, h, w]
    total = probs.sum() + Npix                            # probs.sum() == Npix (+fp noise)
    out = 1 - (intersection + smooth) / (total - intersection + smooth)

Device kernel (per core, one batch item), raw Bass with manual semaphores.
Layout: pixel-partitioned (128, 13, N) chunks, class in the free dim.
Host sends x as bf16 pre-transposed for contiguous chunk DMAs, plus four
u8 range-predicate masks derived from t (a re-encoding of the index
tensor) that drive a 4-instruction blocked mux tree.

Per chunk j (all cross-engine deps via semaphores, no drains):
  ACT : E = exp(X)  (bf16, all 13 classes)
  DVE : A = E[:,0:6]+E[:,6:12]                (denominator tree lvl1)
  DVE : blocked cp-tree on X in place: 13->7->4->2->1  => x_sel = X[:,0]
  GPS : B = A[:,0:3]+A[:,3:6]                 (lvl2)
  DVE : C = (B0+B1)+(B2+E12)                  (lvl3, = per-pixel sum exp)
  ACT : L = ln(C)
  DVE : S = x_sel - L
  ACT : exp(S) with accum_out -> acc[:, j]    (free per-partition reduce)
Host sums acc over cores/partitions/chunks and forms the IoU scalar.
"""

import numpy as np
import ml_dtypes

_BS, _C, _H, _W = 8, 13, 800, 800
_P = 128
_FREE = (_H * _W) // _P  # 5000
_N = 625                 # chunk free size
_NCHUNK = _FREE // _N    # 8
_NBUF = 3
_NCORES = 8
_NPIX = _BS * _H * _W    # 5120000

_cached = {}


def _build_program():
    from contextlib import ExitStack

    import concourse.bass as bass
    import concourse.mybir as mybir

    f32 = mybir.dt.float32
    bf16 = mybir.dt.bfloat16
    u8 = mybir.dt.uint8
    Alu = mybir.AluOpType
    Act = mybir.ActivationFunctionType

    nc = bass.Bass(trn_type="TRN2")
    x = nc.declare_dram_parameter("x", [_P, _NCHUNK, _C, _N], bf16,
                                  isOutput=False)
    m = nc.declare_dram_parameter("m", [_P, 4, _FREE], u8, isOutput=False)
    part = nc.declare_dram_parameter("part", [_P, _NCHUNK], f32, isOutput=True)

    ctx = ExitStack()
    with ctx:
        M = ctx.enter_context(nc.sbuf_tensor("M", [_P, 4, _FREE], u8))
        X = [ctx.enter_context(nc.sbuf_tensor(f"X{i}", [_P, _C, _N], bf16))
             for i in range(_NBUF)]
        E = [ctx.enter_context(nc.sbuf_tensor(f"E{i}", [_P, _C, _N], bf16))
             for i in range(_NBUF)]
        A = [ctx.enter_context(nc.sbuf_tensor(f"A{i}", [_P, 6, _N], bf16))
             for i in range(_NBUF)]
        B = [ctx.enter_context(nc.sbuf_tensor(f"B{i}", [_P, 3, _N], bf16))
             for i in range(_NBUF)]
        C1 = ctx.enter_context(nc.sbuf_tensor("C1", [_P, _N], bf16))
        C2 = ctx.enter_context(nc.sbuf_tensor("C2", [_P, _N], bf16))
        CD = [ctx.enter_context(nc.sbuf_tensor(f"CD{i}", [_P, _N], bf16))
              for i in range(_NBUF)]
        L = [ctx.enter_context(nc.sbuf_tensor(f"L{i}", [_P, _N], bf16))
             for i in range(_NBUF)]
        S = [ctx.enter_context(nc.sbuf_tensor(f"S{i}", [_P, _N], bf16))
             for i in range(_NBUF)]
        ED = ctx.enter_context(nc.sbuf_tensor("ED", [_P, _N], bf16))
        acc = ctx.enter_context(nc.sbuf_tensor("acc", [_P, _NCHUNK], f32))

        block = ctx.enter_context(nc.Block())
        dma_m = ctx.enter_context(nc.semaphore("dma_m"))
        dma_x = [ctx.enter_context(nc.semaphore(f"dma_x{i}"))
                 for i in range(_NBUF)]
        dma_out = ctx.enter_context(nc.semaphore("dma_out"))
        s_exp = ctx.enter_context(nc.semaphore("s_exp"))
        s_l1 = ctx.enter_context(nc.semaphore("s_l1"))
        s_l2 = ctx.enter_context(nc.semaphore("s_l2"))
        s_C = ctx.enter_context(nc.semaphore("s_C"))
        s_ln = ctx.enter_context(nc.semaphore("s_ln"))
        s_sub = ctx.enter_context(nc.semaphore("s_sub"))
        s_fin = ctx.enter_context(nc.semaphore("s_fin"))

        @block.sync
        def _(sync):
            sync.dma_start(out=M[:, :, :], in_=m[:, :, :]).then_inc(dma_m, 16)
            for j in range(_NCHUNK):
                b = j % _NBUF
                rnd = j // _NBUF
                if j >= _NBUF:
                    # X[b]'s last reader is sub of chunk j-NBUF
                    sync.wait_ge(s_sub, j - _NBUF + 1)
                    sync.wait_ge(dma_x[b], 16 * rnd)
                sync.dma_start(
                    out=X[b][:, :, :], in_=x[:, j, :, :]
                ).then_inc(dma_x[b], 16)
            sync.wait_ge(s_fin, _NCHUNK)
            sync.dma_start(out=part[:, :], in_=acc[:, :]).then_inc(dma_out, 16)
            sync.wait_ge(dma_out, 16)

        @block.scalar
        def _(scalar):
            def ln_of(k):
                bk = k % _NBUF
                scalar.wait_ge(s_C, k + 1)
                if k >= _NBUF:
                    # L[bk] read by sub of chunk k-NBUF
                    scalar.wait_ge(s_sub, k - _NBUF + 1)
                scalar.activation(
                    out=L[bk][:, :], in_=CD[bk][:, :], func=Act.Ln
                ).then_inc(s_ln, 1)

            def expacc_of(k):
                bk = k % _NBUF
                scalar.wait_ge(s_sub, k + 1)
                scalar.activation(
                    out=ED[:, :], in_=S[bk][:, :], func=Act.Exp,
                    accum_out=acc[:, k:k + 1],
                ).then_inc(s_fin, 1)

            for j in range(_NCHUNK):
                b = j % _NBUF
                scalar.wait_ge(dma_x[b], 16 * (j // _NBUF + 1))
                if j >= _NBUF:
                    # E[b] fully consumed by C-ops of chunk j-NBUF
                    scalar.wait_ge(s_C, j - _NBUF + 1)
                scalar.activation(
                    out=E[b][:, :, :], in_=X[b][:, :, :], func=Act.Exp
                ).then_inc(s_exp, 1)
                if j >= 1:
                    ln_of(j - 1)
                if j >= 2:
                    expacc_of(j - 2)
            ln_of(_NCHUNK - 1)
            expacc_of(_NCHUNK - 2)
            expacc_of(_NCHUNK - 1)

        @block.gpsimd
        def _(gpsimd):
            for j in range(_NCHUNK):
                b = j % _NBUF
                gpsimd.wait_ge(s_l1, j + 1)
                if j >= _NBUF:
                    # B[b] read by C-ops of chunk j-NBUF
                    gpsimd.wait_ge(s_C, j - _NBUF + 1)
                gpsimd.tensor_tensor(
                    out=B[b][:, :, :], in0=A[b][:, 0:3, :],
                    in1=A[b][:, 3:6, :], op=Alu.add,
                ).then_inc(s_l2, 1)

        @block.vector
        def _(vector):
            vector.wait_ge(dma_m, 16)

            def sub_of(k):
                bk = k % _NBUF
                vector.wait_ge(s_ln, k + 1)
                if k >= _NBUF:
                    # S[bk] read by expacc of chunk k-NBUF
                    vector.wait_ge(s_fin, k - _NBUF + 1)
                vector.tensor_tensor(
                    out=S[bk][:, :], in0=X[bk][:, 0, :], in1=L[bk][:, :],
                    op=Alu.subtract,
                ).then_inc(s_sub, 1)

            for j in range(_NCHUNK):
                b = j % _NBUF
                vector.wait_ge(s_exp, j + 1)
                if j >= _NBUF:
                    # A[b] read by gpsimd lvl2 of chunk j-NBUF
                    vector.wait_ge(s_l2, j - _NBUF + 1)
                # denominator lvl1: A = E[:,0:6]+E[:,6:12]
                vector.tensor_tensor(
                    out=A[b][:, :, :], in0=E[b][:, 0:6, :],
                    in1=E[b][:, 6:12, :], op=Alu.add,
                ).then_inc(s_l1, 1)
                # blocked mux tree on X (in place): 13 -> 7 -> 4 -> 2 -> 1
                sl = slice(j * _N, (j + 1) * _N)
                ma = M[:, 0, sl].unsqueeze(1)
                vector.copy_predicated(
                    X[b][:, 0:6, :], ma.broadcast_to((_P, 6, _N)),
                    X[b][:, 7:13, :])
                mb = M[:, 1, sl].unsqueeze(1)
                vector.copy_predicated(
                    X[b][:, 0:3, :], mb.broadcast_to((_P, 3, _N)),
                    X[b][:, 4:7, :])
                mc = M[:, 2, sl].unsqueeze(1)
                vector.copy_predicated(
                    X[b][:, 0:2, :], mc.broadcast_to((_P, 2, _N)),
                    X[b][:, 2:4, :])
                md = M[:, 3, sl].unsqueeze(1)
                vector.copy_predicated(
                    X[b][:, 0:1, :], md.broadcast_to((_P, 1, _N)),
                    X[b][:, 1:2, :])
                # denominator lvl3
                vector.wait_ge(s_l2, j + 1)
                if j >= _NBUF:
                    # CD[b] read by ln of chunk j-NBUF
                    vector.wait_ge(s_ln, j - _NBUF + 1)
                vector.tensor_tensor(out=C1[:, :], in0=B[b][:, 0, :],
                                     in1=B[b][:, 1, :], op=Alu.add)
                vector.tensor_tensor(out=C2[:, :], in0=B[b][:, 2, :],
                                     in1=E[b][:, 12, :], op=Alu.add)
                vector.tensor_tensor(
                    out=CD[b][:, :], in0=C1[:, :], in1=C2[:, :], op=Alu.add,
                ).then_inc(s_C, 1)
                if j >= 1:
                    sub_of(j - 1)
            sub_of(_NCHUNK - 1)

    return nc


def _get_program():
    if "nc" not in _cached:
        _cached["nc"] = _build_program()
    return _cached["nc"]


def _make_in_maps(inputs, targets):
    in_maps = []
    for b in range(_NCORES):
        xb = np.asarray(inputs[b]).reshape(_C, _P, _FREE)
        # (128, NCHUNK, 13, N) so each chunk is contiguous per partition
        xh = np.ascontiguousarray(
            xb.transpose(1, 0, 2).reshape(_P, _C, _NCHUNK, _N)
            .transpose(0, 2, 1, 3)
        ).astype(ml_dtypes.bfloat16)
        t = np.asarray(targets[b]).astype(np.int64).reshape(_P, _FREE)
        # blocked mux-tree predicates (pure index re-encoding of t)
        ma = t >= 7
        t1 = t - 7 * ma
        mb = t1 >= 4
        t2 = t1 - 4 * mb
        mc = t2 >= 2
        t3 = t2 - 2 * mc
        md = t3 >= 1
        mh = np.ascontiguousarray(
            np.stack([ma, mb, mc, md], axis=1).astype(np.uint8))
        in_maps.append({"x": xh, "m": mh})
    return in_maps


def _finalize(parts, smooth):
    inter = 0.0
    for p in parts:
        inter += float(np.sum(np.asarray(p).astype(np.float64)))
    s = float(smooth)
    total = 2.0 * float(_NPIX)
    union = total - inter
    out = 1.0 - (inter + s) / (union + s)
    return np.asarray(np.float32(out))


def kernel(inputs, targets, smooth):
    from concourse.bass_utils import run_bass_kernel_spmd

    nc = _get_program()
    in_maps = _make_in_maps(np.asarray(inputs), np.asarray(targets))
    res = run_bass_kernel_spmd(nc, in_maps, list(range(_NCORES)))
    return _finalize([res.results[b]["part"] for b in range(_NCORES)], smooth)


# revision 5
# speedup vs baseline: 1.0539x; 1.0539x over previous
"""IoU loss kernel for Trainium2, data-parallel over the batch dim on 8 cores.

Math (per reference):
    probs = softmax(inputs, axis=1)                       # (8, 13, 800, 800)
    intersection = sum_pix probs[b, t, h, w]
    total = probs.sum() + Npix                            # probs.sum() == Npix (+fp noise)
    out = 1 - (intersection + smooth) / (total - intersection + smooth)

Device kernel (per core, one batch item), raw Bass with manual semaphores.
Layout: pixel-partitioned (128, 13, N) chunks, class in the free dim.
Host sends x as bf16 pre-transposed for contiguous chunk DMAs, plus four
u8 range-predicate masks derived from t (a re-encoding of the index
tensor) that drive a 4-instruction blocked mux tree.

Per chunk j (all cross-engine deps via semaphores, no drains):
  ACT : E = exp(X)  (bf16, all 13 classes)
  DVE : A = E[:,0:6]+E[:,6:12]                (denominator tree lvl1)
  DVE : blocked cp-tree on X in place: 13->7->4->2->1  => x_sel = X[:,0]
  GPS : B = A[:,0:3]+A[:,3:6]                 (lvl2)
  DVE : C = (B0+B1)+(B2+E12)                  (lvl3, = per-pixel sum exp)
  ACT : L = ln(C)
  DVE : S = x_sel - L
  ACT : exp(S) with accum_out -> acc[:, j]    (free per-partition reduce)
Host sums acc over cores/partitions/chunks and forms the IoU scalar.
"""

import numpy as np
import ml_dtypes

_BS, _C, _H, _W = 8, 13, 800, 800
_P = 128
_FREE = (_H * _W) // _P  # 5000
_N = 625                 # chunk free size
_NCHUNK = _FREE // _N    # 8
_NBUF = 3
_NCORES = 8
_NPIX = _BS * _H * _W    # 5120000

_cached = {}


def _build_program():
    from contextlib import ExitStack

    import concourse.bass as bass
    import concourse.mybir as mybir

    f32 = mybir.dt.float32
    bf16 = mybir.dt.bfloat16
    u8 = mybir.dt.uint8
    Alu = mybir.AluOpType
    Act = mybir.ActivationFunctionType

    nc = bass.Bass(trn_type="TRN2")
    x = nc.declare_dram_parameter("x", [_P, _NCHUNK, _C, _N], bf16,
                                  isOutput=False)
    m = nc.declare_dram_parameter("m", [_P, 4, _FREE], u8, isOutput=False)
    part = nc.declare_dram_parameter("part", [_P, _NCHUNK], f32, isOutput=True)

    ctx = ExitStack()
    with ctx:
        M = ctx.enter_context(nc.sbuf_tensor("M", [_P, 4, _FREE], u8))
        X = [ctx.enter_context(nc.sbuf_tensor(f"X{i}", [_P, _C, _N], bf16))
             for i in range(_NBUF)]
        E = [ctx.enter_context(nc.sbuf_tensor(f"E{i}", [_P, _C, _N], bf16))
             for i in range(_NBUF)]
        A = [ctx.enter_context(nc.sbuf_tensor(f"A{i}", [_P, 6, _N], bf16))
             for i in range(_NBUF)]
        B = [ctx.enter_context(nc.sbuf_tensor(f"B{i}", [_P, 3, _N], bf16))
             for i in range(_NBUF)]
        C1 = ctx.enter_context(nc.sbuf_tensor("C1", [_P, _N], bf16))
        C2 = ctx.enter_context(nc.sbuf_tensor("C2", [_P, _N], bf16))
        CD = [ctx.enter_context(nc.sbuf_tensor(f"CD{i}", [_P, _N], bf16))
              for i in range(_NBUF)]
        L = [ctx.enter_context(nc.sbuf_tensor(f"L{i}", [_P, _N], bf16))
             for i in range(_NBUF)]
        S = [ctx.enter_context(nc.sbuf_tensor(f"S{i}", [_P, _N], bf16))
             for i in range(_NBUF)]
        ED = ctx.enter_context(nc.sbuf_tensor("ED", [_P, _N], bf16))
        acc = ctx.enter_context(nc.sbuf_tensor("acc", [_P, _NCHUNK], f32))

        block = ctx.enter_context(nc.Block())
        dma_m = ctx.enter_context(nc.semaphore("dma_m"))
        dma_x = [ctx.enter_context(nc.semaphore(f"dma_x{i}"))
                 for i in range(_NBUF)]
        dma_out = ctx.enter_context(nc.semaphore("dma_out"))
        s_exp = ctx.enter_context(nc.semaphore("s_exp"))
        s_l1 = ctx.enter_context(nc.semaphore("s_l1"))
        s_l2 = ctx.enter_context(nc.semaphore("s_l2"))
        s_C = ctx.enter_context(nc.semaphore("s_C"))
        s_ln = ctx.enter_context(nc.semaphore("s_ln"))
        s_sub = ctx.enter_context(nc.semaphore("s_sub"))
        s_fin = ctx.enter_context(nc.semaphore("s_fin"))

        @block.sync
        def _(sync):
            for j in range(_NCHUNK):
                b = j % _NBUF
                rnd = j // _NBUF
                if j >= _NBUF:
                    # X[b]'s last reader is sub of chunk j-NBUF
                    sync.wait_ge(s_sub, j - _NBUF + 1)
                    sync.wait_ge(dma_x[b], 16 * rnd)
                sync.dma_start(
                    out=X[b][:, :, :], in_=x[:, j, :, :]
                ).then_inc(dma_x[b], 16)
                if j == 0:
                    # masks are first needed by the cp tree of chunk 0,
                    # which runs after exp(0); x(0) goes first so the
                    # scalar engine starts as early as possible.
                    sync.dma_start(out=M[:, :, :],
                                   in_=m[:, :, :]).then_inc(dma_m, 16)
            sync.wait_ge(s_fin, _NCHUNK)
            sync.dma_start(out=part[:, :], in_=acc[:, :]).then_inc(dma_out, 16)
            sync.wait_ge(dma_out, 16)

        @block.scalar
        def _(scalar):
            def ln_of(k):
                bk = k % _NBUF
                scalar.wait_ge(s_C, k + 1)
                if k >= _NBUF:
                    # L[bk] read by sub of chunk k-NBUF
                    scalar.wait_ge(s_sub, k - _NBUF + 1)
                scalar.activation(
                    out=L[bk][:, :], in_=CD[bk][:, :], func=Act.Ln
                ).then_inc(s_ln, 1)

            def expacc_of(k):
                bk = k % _NBUF
                scalar.wait_ge(s_sub, k + 1)
                scalar.activation(
                    out=ED[:, :], in_=S[bk][:, :], func=Act.Exp,
                    accum_out=acc[:, k:k + 1],
                ).then_inc(s_fin, 1)

            for j in range(_NCHUNK):
                b = j % _NBUF
                scalar.wait_ge(dma_x[b], 16 * (j // _NBUF + 1))
                if j >= _NBUF:
                    # E[b] fully consumed by C-ops of chunk j-NBUF
                    scalar.wait_ge(s_C, j - _NBUF + 1)
                scalar.activation(
                    out=E[b][:, :, :], in_=X[b][:, :, :], func=Act.Exp
                ).then_inc(s_exp, 1)
                if j >= 1:
                    ln_of(j - 1)
                if j >= 2:
                    expacc_of(j - 2)
            ln_of(_NCHUNK - 1)
            expacc_of(_NCHUNK - 2)
            expacc_of(_NCHUNK - 1)

        @block.gpsimd
        def _(gpsimd):
            for j in range(_NCHUNK):
                b = j % _NBUF
                gpsimd.wait_ge(s_l1, j + 1)
                if j >= _NBUF:
                    # B[b] read by C-ops of chunk j-NBUF
                    gpsimd.wait_ge(s_C, j - _NBUF + 1)
                gpsimd.tensor_tensor(
                    out=B[b][:, :, :], in0=A[b][:, 0:3, :],
                    in1=A[b][:, 3:6, :], op=Alu.add,
                ).then_inc(s_l2, 1)

        @block.vector
        def _(vector):
            vector.wait_ge(dma_m, 16)

            def sub_of(k):
                bk = k % _NBUF
                vector.wait_ge(s_ln, k + 1)
                if k >= _NBUF:
                    # S[bk] read by expacc of chunk k-NBUF
                    vector.wait_ge(s_fin, k - _NBUF + 1)
                vector.tensor_tensor(
                    out=S[bk][:, :], in0=X[bk][:, 0, :], in1=L[bk][:, :],
                    op=Alu.subtract,
                ).then_inc(s_sub, 1)

            for j in range(_NCHUNK):
                b = j % _NBUF
                vector.wait_ge(s_exp, j + 1)
                if j >= _NBUF:
                    # A[b] read by gpsimd lvl2 of chunk j-NBUF
                    vector.wait_ge(s_l2, j - _NBUF + 1)
                # denominator lvl1: A = E[:,0:6]+E[:,6:12]
                vector.tensor_tensor(
                    out=A[b][:, :, :], in0=E[b][:, 0:6, :],
                    in1=E[b][:, 6:12, :], op=Alu.add,
                ).then_inc(s_l1, 1)
                # blocked mux tree on X (in place): 13 -> 7 -> 4 -> 2 -> 1.
                # Each level is split into two free-dim halves: the second
                # half of level k is independent of the first half of level
                # k+1, so it hides the predicated-write pipeline drain that
                # a dependent cp would otherwise stall on.
                NH = _N // 2
                tree = [(0, 6, 7), (1, 3, 4), (2, 2, 2), (3, 1, 1)]
                for lev, width, off in tree:
                    for h in range(2):
                        fsl = slice(h * NH, (h + 1) * NH)
                        msl = slice(j * _N + h * NH, j * _N + (h + 1) * NH)
                        mk = M[:, lev, msl].unsqueeze(1)
                        vector.copy_predicated(
                            X[b][:, 0:width, fsl],
                            mk.broadcast_to((_P, width, NH)),
                            X[b][:, off:off + width, fsl])
                # denominator lvl3
                vector.wait_ge(s_l2, j + 1)
                if j >= _NBUF:
                    # CD[b] read by ln of chunk j-NBUF
                    vector.wait_ge(s_ln, j - _NBUF + 1)
                vector.tensor_tensor(out=C1[:, :], in0=B[b][:, 0, :],
                                     in1=B[b][:, 1, :], op=Alu.add)
                vector.tensor_tensor(out=C2[:, :], in0=B[b][:, 2, :],
                                     in1=E[b][:, 12, :], op=Alu.add)
                vector.tensor_tensor(
                    out=CD[b][:, :], in0=C1[:, :], in1=C2[:, :], op=Alu.add,
                ).then_inc(s_C, 1)
                if j >= 1:
                    sub_of(j - 1)
            sub_of(_NCHUNK - 1)

    return nc


def _get_program():
    if "nc" not in _cached:
        _cached["nc"] = _build_program()
    return _cached["nc"]


def _make_in_maps(inputs, targets):
    in_maps = []
    for b in range(_NCORES):
        xb = np.asarray(inputs[b]).reshape(_C, _P, _FREE)
        # (128, NCHUNK, 13, N) so each chunk is contiguous per partition
        xh = np.ascontiguousarray(
            xb.transpose(1, 0, 2).reshape(_P, _C, _NCHUNK, _N)
            .transpose(0, 2, 1, 3)
        ).astype(ml_dtypes.bfloat16)
        t = np.asarray(targets[b]).astype(np.int64).reshape(_P, _FREE)
        # blocked mux-tree predicates (pure index re-encoding of t)
        ma = t >= 7
        t1 = t - 7 * ma
        mb = t1 >= 4
        t2 = t1 - 4 * mb
        mc = t2 >= 2
        t3 = t2 - 2 * mc
        md = t3 >= 1
        mh = np.ascontiguousarray(
            np.stack([ma, mb, mc, md], axis=1).astype(np.uint8))
        in_maps.append({"x": xh, "m": mh})
    return in_maps


def _finalize(parts, smooth):
    inter = 0.0
    for p in parts:
        inter += float(np.sum(np.asarray(p).astype(np.float64)))
    s = float(smooth)
    total = 2.0 * float(_NPIX)
    union = total - inter
    out = 1.0 - (inter + s) / (union + s)
    return np.asarray(np.float32(out))


def kernel(inputs, targets, smooth):
    from concourse.bass_utils import run_bass_kernel_spmd

    nc = _get_program()
    in_maps = _make_in_maps(np.asarray(inputs), np.asarray(targets))
    res = run_bass_kernel_spmd(nc, in_maps, list(range(_NCORES)))
    return _finalize([res.results[b]["part"] for b in range(_NCORES)], smooth)
